# revision 1
# baseline (speedup 1.0000x reference)
"""Trainium2 Bass kernel for nn_ConduitHydrology: CG solve of the 5-point
Neumann Laplacian on a 2048x2048 raster, sharded by row-blocks over 8 cores.

kernel(**inputs) takes FULL inputs and returns the FULL output.
"""
import numpy as np

import concourse.bass as bass
import concourse.bacc as bacc
import concourse.mybir as mybir
import concourse.tile as tile
from concourse.bass_utils import run_bass_kernel_spmd

F32 = mybir.dt.float32
I32 = mybir.dt.int32
NCORES = 8
R, C = 2048, 2048
BR = R // NCORES          # 256 rows per core
W = C                     # 2048 cols
HALF = 2048
NFREE = 2 * HALF          # [128, 4096] per state tile
NITER = 100
DX = 100.0
CHUNK = 512
SKIP_CC = False
SKIP_DMA = False

_compiled = None


def _build_program(niter):
    nc = bacc.Bacc("TRN2", target_bir_lowering=False, debug=False,
                   num_devices=NCORES)

    b_in = nc.dram_tensor("bblk", [128, NFREE], F32, kind="ExternalInput")
    t0_in = nc.dram_tensor("t0", [128, 128], F32, kind="ExternalInput")
    t1_in = nc.dram_tensor("t1", [128, 128], F32, kind="ExternalInput")
    u0_in = nc.dram_tensor("u0", [2, 128], F32, kind="ExternalInput")
    u1_in = nc.dram_tensor("u1", [2, 128], F32, kind="ExternalInput")
    gp_in = nc.dram_tensor("gp0", [2, HALF], F32, kind="ExternalInput")
    gidx_in = nc.dram_tensor("gidx", [2, 1], I32, kind="ExternalInput")
    gam_in = nc.dram_tensor("gam0", [1, 1], F32, kind="ExternalInput")
    x_out = nc.dram_tensor("xout", [128, NFREE], F32, kind="ExternalOutput")

    ccA_in = nc.dram_tensor("ccA_in", [1, 16], F32, kind="Internal")
    ccA_out = nc.dram_tensor("ccA_out", [NCORES, 16], F32, kind="Internal",
                             addr_space="Shared")
    ccB_in = nc.dram_tensor("ccB_in", [3, HALF], F32, kind="Internal")
    ccB_out = nc.dram_tensor("ccB_out", [3 * NCORES, HALF], F32,
                             kind="Internal", addr_space="Shared")
    rg = [list(range(NCORES))]

    with tile.TileContext(nc) as tc:
        with tc.tile_pool(name="state", bufs=1) as sp, \
             tc.tile_pool(name="psumV", bufs=1, space="PSUM") as ppv, \
             tc.tile_pool(name="psumS", bufs=1, space="PSUM") as pps:
            # persistent state
            x = sp.tile([128, NFREE], F32, name="x")
            r = sp.tile([128, NFREE], F32, name="r")
            p = sp.tile([128, NFREE], F32, name="p")
            q = sp.tile([128, NFREE], F32, name="q")
            s1 = sp.tile([128, NFREE], F32, name="s1")
            t0 = sp.tile([128, 128], F32, name="t0s")
            t1 = sp.tile([128, 128], F32, name="t1s")
            u0 = sp.tile([2, 128], F32, name="u0s")
            u1 = sp.tile([2, 128], F32, name="u1s")
            aux0 = sp.tile([2, HALF], F32, name="aux0")
            aux1 = sp.tile([2, HALF], F32, name="aux1")
            gp = sp.tile([2, HALF], F32, name="gp")
            rgp = sp.tile([2, HALF], F32, name="rgp")
            gidx = sp.tile([2, 1], I32, name="gidx")
            gam = sp.tile([1, 1], F32, name="gam")
            ones_c = sp.tile([128, 1], F32, name="ones_c")
            ones_r = sp.tile([1, 128], F32, name="ones_r")
            pq_part = sp.tile([128, 1], F32, name="pq_part")
            rr_part = sp.tile([128, 1], F32, name="rr_part")
            sqd = sp.tile([128, NFREE], F32, name="sqd")
            sqd2 = sp.tile([128, NFREE], F32, name="sqd2")
            g8 = sp.tile([1, 128], F32, name="g8")
            sc = sp.tile([1, 8], F32, name="sc")  # scalar scratch
            ab = sp.tile([128, 2], F32, name="ab")  # alpha / -alpha bcast
            bb = sp.tile([128, 1], F32, name="bb")  # beta bcast

            # ---- init ----
            nc.sync.dma_start(r[:], b_in.ap())
            nc.sync.dma_start(p[:], b_in.ap())
            nc.sync.dma_start(t0[:], t0_in.ap())
            nc.sync.dma_start(t1[:], t1_in.ap())
            nc.sync.dma_start(u0[:], u0_in.ap())
            nc.sync.dma_start(u1[:], u1_in.ap())
            nc.sync.dma_start(gp[:], gp_in.ap())
            nc.sync.dma_start(gidx[:], gidx_in.ap())
            nc.sync.dma_start(gam[:], gam_in.ap())
            nc.vector.memset(x[:], 0.0)
            nc.vector.memset(ones_c[:], 1.0)
            nc.vector.memset(ones_r[:], 1.0)
            nc.vector.memset(g8[:], 1.0)
            # initial aux: ghost rows + local cross-slab rows of p(=b)
            nc.sync.dma_start(aux0[0:1, :], gp_in.ap()[0:1, :])
            nc.sync.dma_start(aux1[1:2, :], gp_in.ap()[1:2, :])
            nc.sync.dma_start(aux0[1:2, :], b_in.ap()[0:1, HALF:NFREE])
            nc.sync.dma_start(aux1[0:1, :], b_in.ap()[127:128, 0:HALF])

            ts = [t0, t1]
            us = [u0, u1]
            auxs = [aux0, aux1]

            for it in range(niter):
                # ---- matvec q = L p ----
                for s in range(2):
                    o = s * HALF
                    ps = p[:, o:o + HALF]
                    # horizontal shifted sums with edge-column degh fix
                    nc.vector.tensor_tensor(
                        s1[:, o + 1:o + HALF - 1], ps[:, 0:HALF - 2],
                        ps[:, 2:HALF], mybir.AluOpType.add)
                    nc.vector.tensor_tensor(
                        s1[:, o:o + 1], ps[:, 0:1], ps[:, 1:2],
                        mybir.AluOpType.add)
                    nc.vector.tensor_tensor(
                        s1[:, o + HALF - 1:o + HALF], ps[:, HALF - 2:HALF - 1],
                        ps[:, HALF - 1:HALF], mybir.AluOpType.add)
                    # vertical + diagonal via PE
                    vt = ppv.tile([128, HALF], F32, name="vt", tag="vt")
                    for ch in range(0, HALF, CHUNK):
                        nc.tensor.matmul(vt[:, ch:ch + CHUNK], ts[s][:],
                                         ps[:, ch:ch + CHUNK],
                                         start=True, stop=False)
                        nc.tensor.matmul(vt[:, ch:ch + CHUNK], us[s][:],
                                         auxs[s][:, ch:ch + CHUNK],
                                         start=False, stop=True)
                    nc.vector.tensor_tensor(
                        q[:, o:o + HALF], s1[:, o:o + HALF], vt[:],
                        mybir.AluOpType.add)

                # ---- pq = p . q (accum per partition, then partition reduce)
                nc.vector.scalar_tensor_tensor(
                    sqd[:], p[:], 1.0, q[:],
                    mybir.AluOpType.mult, mybir.AluOpType.mult,
                    accum_out=pq_part[:])
                red = pps.tile([1, 2], F32, name="red", tag="red")
                nc.tensor.matmul(red[:, 0:1], ones_c[:], pq_part[:],
                                 start=True, stop=True)
                nc.scalar.copy(sc[:, 0:1], red[:, 0:1])
                if not SKIP_DMA:
                    nc.sync.dma_start(ccA_in.ap()[0:1, 0:1], sc[:, 0:1])
                if not SKIP_CC:
                    nc.gpsimd.collective_compute(
                        "AllGather", mybir.AluOpType.bypass, replica_groups=rg,
                        ins=[ccA_in.ap()], outs=[ccA_out.ap()])
                if not SKIP_DMA:
                    nc.sync.dma_start(
                        g8[:], ccA_out.ap().rearrange("(o a) b -> o (a b)", o=1))
                nc.vector.tensor_reduce(
                    sc[:, 1:2],
                    g8[0:1, :].rearrange("a (c s) -> a c s", s=16)[:, :, 0:1],
                    axis=mybir.AxisListType.XY, op=mybir.AluOpType.add)
                # alpha = gam / pq ; nalpha = -alpha
                nc.vector.reciprocal(sc[:, 2:3], sc[:, 1:2])
                nc.vector.tensor_tensor(sc[:, 3:4], sc[:, 2:3], gam[:],
                                        mybir.AluOpType.mult)
                nc.vector.tensor_scalar_mul(sc[:, 4:5], sc[:, 3:4], -1.0)
                bc = pps.tile([128, 2], F32, name="bc", tag="bc")
                nc.tensor.matmul(bc[:], ones_r[:], sc[0:1, 3:5],
                                 start=True, stop=True)
                nc.scalar.copy(ab[:], bc[:])

                # ---- r -= alpha q ----
                nc.vector.scalar_tensor_tensor(
                    r[:], q[:], ab[:, 1:2], r[:],
                    mybir.AluOpType.mult, mybir.AluOpType.add)

                # ---- rr partial on ACT; stage CC-B (rr + r boundary rows)
                nc.scalar.activation(sqd2[:], r[:],
                                     mybir.ActivationFunctionType.Square,
                                     accum_out=rr_part[:])
                red2 = pps.tile([1, 2], F32, name="red2", tag="red")
                nc.tensor.matmul(red2[:, 0:1], ones_c[:], rr_part[:],
                                 start=True, stop=True)
                nc.scalar.copy(sc[:, 5:6], red2[:, 0:1])
                if not SKIP_DMA:
                    nc.sync.dma_start(ccB_in.ap()[0:1, 0:1], sc[:, 5:6])
                    nc.sync.dma_start(ccB_in.ap()[1:2, :], r[0:1, 0:HALF])
                    nc.sync.dma_start(ccB_in.ap()[2:3, :], r[127:128, HALF:NFREE])
                if not SKIP_CC:
                    nc.gpsimd.collective_compute(
                        "AllGather", mybir.AluOpType.bypass, replica_groups=rg,
                        ins=[ccB_in.ap()], outs=[ccB_out.ap()])

                # ---- x += alpha p (overlaps CC-B) ----
                nc.vector.scalar_tensor_tensor(
                    x[:], p[:], ab[:, 0:1], x[:],
                    mybir.AluOpType.mult, mybir.AluOpType.add)

                # ---- gamma_new, beta ----
                if not SKIP_DMA:
                    nc.sync.dma_start(
                        g8[:, 0:24],
                        ccB_out.ap()[:, 0:1].rearrange("(o a) b -> o (a b)", o=1))
                nc.vector.tensor_reduce(
                    sc[:, 6:7],
                    g8[0:1, 0:24].rearrange("a (c s) -> a c s", s=3)[:, :, 0:1],
                    axis=mybir.AxisListType.XY, op=mybir.AluOpType.add)
                nc.vector.reciprocal(sc[:, 7:8], gam[:])
                nc.vector.tensor_tensor(sc[:, 0:1], sc[:, 6:7], sc[:, 7:8],
                                        mybir.AluOpType.mult)
                nc.vector.tensor_copy(gam[:], sc[:, 6:7])
                bc2 = pps.tile([128, 1], F32, name="bc2", tag="bc")
                nc.tensor.matmul(bc2[:], ones_r[:], sc[0:1, 0:1],
                                 start=True, stop=True)
                nc.scalar.copy(bb[:], bc2[:])

                # ---- p = r + beta p ----
                nc.vector.scalar_tensor_tensor(
                    p[:], p[:], bb[:], r[:],
                    mybir.AluOpType.mult, mybir.AluOpType.add)

                if it < niter - 1 and not SKIP_DMA:
                    # ---- ghost p rows: gp = beta*gp + r_ghost ----
                    nc.gpsimd.indirect_dma_start(
                        out=rgp[:], out_offset=None, in_=ccB_out.ap(),
                        in_offset=bass.IndirectOffsetOnAxis(ap=gidx[:, :1],
                                                            axis=0))
                    nc.vector.scalar_tensor_tensor(
                        gp[:], gp[:], bb[0:2, :], rgp[:],
                        mybir.AluOpType.mult, mybir.AluOpType.add)
                    # refresh aux tiles
                    nc.sync.dma_start(aux0[0:1, :], gp[0:1, :])
                    nc.sync.dma_start(aux1[1:2, :], gp[1:2, :])
                    nc.sync.dma_start(aux0[1:2, :], p[0:1, HALF:NFREE])
                    nc.sync.dma_start(aux1[0:1, :], p[127:128, 0:HALF])

            nc.sync.dma_start(x_out.ap(), x[:])

    nc.compile()
    return nc


def _host_prep(conduit_size, discharge, geometric_gradient, nrows, ncols):
    cs = np.asarray(conduit_size, dtype=np.float32).reshape(R, C)
    dc = np.asarray(discharge, dtype=np.float32).reshape(R, C)
    gg = np.asarray(geometric_gradient, dtype=np.float32).reshape(R, C)

    gn = (dc * np.float32(0.0405) * cs ** np.float32(1.25)) ** 2
    gn = gn.astype(np.float32)
    gh = np.float32(0.5) * (gn[:, :-1] + gn[:, 1:])
    gv = np.float32(0.5) * (gn[:-1, :] + gn[1:, :])
    rr_ = np.arange(R)[:, None]
    cc_ = np.arange(C)[None, :]
    status = (rr_ == 0) | (rr_ == R - 1) | (cc_ == 0) | (cc_ == C - 1)
    ih = status[:, :-1] | status[:, 1:]
    iv = status[:-1, :] | status[1:, :]
    ggh = np.float32(0.5) * (gg[:, :-1] + gg[:, 1:])
    ggv = np.float32(0.5) * (gg[:-1, :] + gg[1:, :])
    gh = np.where(ih, ggh, gh).astype(np.float32)
    gv = np.where(iv, ggv, gv).astype(np.float32)
    b = np.zeros((R, C), dtype=np.float32)
    b[:, :-1] += gh
    b[:, 1:] -= gh
    b[:-1, :] += gv
    b[1:, :] -= gv
    return b, gg


def _pack(blk):
    """[256, 2048] core block -> [128, 4096] (slab0 | slab1)."""
    return np.concatenate([blk[0:128, :], blk[128:256, :]], axis=1)


def _unpack(t):
    return np.concatenate([t[:, 0:HALF], t[:, HALF:NFREE]], axis=0)


def kernel(conduit_size, discharge, geometric_gradient, nrows, ncols):
    global _compiled
    b, gg = _host_prep(conduit_size, discharge, geometric_gradient,
                       nrows, ncols)

    if _compiled is None:
        _compiled = _build_program(NITER)
    nc = _compiled

    gamma0 = np.float32(np.dot(b.ravel(), b.ravel()))

    in_maps = []
    for i in range(NCORES):
        blk = b[i * BR:(i + 1) * BR, :]
        # T matrices: tridiagonal, diag = -(degv+2)
        t0 = np.zeros((128, 128), dtype=np.float32)
        t1 = np.zeros((128, 128), dtype=np.float32)
        for t, base in ((t0, i * BR), (t1, i * BR + 128)):
            for j in range(128):
                grow = base + j
                degv = 2 - (1 if grow == 0 else 0) - (1 if grow == R - 1 else 0)
                t[j, j] = -(degv + 2)
                if j > 0:
                    t[j, j - 1] = 1.0
                if j < 127:
                    t[j, j + 1] = 1.0
        u0 = np.zeros((2, 128), dtype=np.float32)
        u1 = np.zeros((2, 128), dtype=np.float32)
        u0[0, 0] = 0.0 if i == 0 else 1.0
        u0[1, 127] = 1.0
        u1[0, 0] = 1.0
        u1[1, 127] = 0.0 if i == NCORES - 1 else 1.0
        # initial p ghosts (p0 = b)
        gp0 = np.zeros((2, HALF), dtype=np.float32)
        if i > 0:
            gp0[0, :] = b[i * BR - 1, :]
        if i < NCORES - 1:
            gp0[1, :] = b[(i + 1) * BR, :]
        # r-ghost gather indices into ccB_out [3*8, HALF]:
        # ghost_bot = last row of core i-1 = its ccB row 2; ghost_top =
        # first row of core i+1 = its ccB row 1. Cores 0/7: point at a
        # harmless finite row (own slot), contribution killed by U mask.
        lo = 3 * (i - 1) + 2 if i > 0 else 1
        hi = 3 * (i + 1) + 1 if i < NCORES - 1 else 1
        gidx = np.array([[lo], [hi]], dtype=np.int32)
        in_maps.append({
            "bblk": _pack(blk), "t0": t0, "t1": t1, "u0": u0, "u1": u1,
            "gp0": gp0, "gidx": gidx,
            "gam0": gamma0.reshape(1, 1),
        })

    res = run_bass_kernel_spmd(nc, in_maps, core_ids=list(range(NCORES)))

    y = np.zeros((R, C), dtype=np.float32)
    for i in range(NCORES):
        y[i * BR:(i + 1) * BR, :] = _unpack(res.results[i]["xout"])

    out = (gg - np.float32(DX) * y).astype(np.float32)
    return out.reshape(-1)



# revision 5
# speedup vs baseline: 1.6264x; 1.6264x over previous
"""Trainium2 Bass kernel for nn_ConduitHydrology: 100 iterations of
single-reduction (Chronopoulos-Gear) CG on the 5-point Neumann Laplacian,
2048x2048 raster, row-blocks over 8 cores, f16 I/O.

kernel(**inputs) takes FULL inputs and returns the FULL output.
"""
import numpy as np

import concourse.bass as bass
import concourse.bacc as bacc
import concourse.mybir as mybir
import concourse.tile as tile
from concourse.bass_utils import run_bass_kernel_spmd
from concourse import bass_isa

F32 = mybir.dt.float32
F16 = mybir.dt.float16
I32 = mybir.dt.int32
NCORES = 8
R, C = 2048, 2048
BR = R // NCORES          # 256 grid rows per core, packed as 2 slabs of 128
HALF = 2048
W = 4104                  # [gg | s0(2048) | gg | s1(2048) | gggg]
NITER = 100
DX = 100.0

_compiled = None


def _int(t, off):
    """Nested AP covering both slab interiors of a width-W region at column
    offset `off` within the tile: cols {off+2..off+2049} u {off+2052..off+4099}."""
    return t[:, off + 2:off + 4102].rearrange("p (a w) -> p a w", a=2)[:, :, 0:2048]


def _intsh(t, off, sh):
    """Interior nested AP shifted by sh (-1 left neighbors, +1 right)."""
    base = off + 2 + sh
    return t[:, base:base + 4100].rearrange("p (a w) -> p a w", a=2)[:, :, 0:2048]


def _edge(t, off):
    """The 4 slab-edge columns {off+2, off+2049, off+2052, off+4099}."""
    return t[:, off + 2:off + 4102].rearrange(
        "p (a w) -> p a w", a=2)[:, :, 0:2048:2047]


def _build_program(niter):
    nc = bacc.Bacc("TRN2", target_bir_lowering=False, debug=False,
                   num_devices=NCORES)

    b_in = nc.dram_tensor("bblk", [128, W], F16, kind="ExternalInput")
    gb_in = nc.dram_tensor("gb", [2, HALF], F16, kind="ExternalInput")
    gidx_in = nc.dram_tensor("gidx", [2, 1], I32, kind="ExternalInput")
    x_out = nc.dram_tensor("xout", [128, 2 * HALF], F16, kind="ExternalOutput")

    cc_in = nc.dram_tensor("cc_in", [3, HALF], F32, kind="Internal")
    cc_out = nc.dram_tensor("cc_out", [3 * NCORES, HALF], F32,
                            kind="Internal", addr_space="Shared")
    rg = [list(range(NCORES))]

    with tile.TileContext(nc) as tc:
        with tc.tile_pool(name="state", bufs=1) as sp, \
             tc.tile_pool(name="psumB", bufs=1, space="PSUM") as ppb:
            # xrs = [x | r | s~] with s~ = -A r; pq = [p | q~], q~ = -A p
            xrs = sp.tile([128, 3 * W], F32, name="xrs")
            pq = sp.tile([128, 2 * W], F32, name="pq")
            s1 = sp.tile([128, W], F32, name="s1")
            tt = sp.tile([128, W], F32, name="tt")
            up = sp.tile([128, W], F32, name="up")
            dn = sp.tile([128, W], F32, name="dn")
            hb = sp.tile([128, W], F16, name="hb")
            of = sp.tile([128, 2 * HALF], F16, name="of")
            g = sp.tile([2, 2 * HALF], F32, name="g")      # [gq~ | gr]
            gs = sp.tile([2, HALF], F32, name="gs")        # ghost s~ rows
            gh16 = sp.tile([2, HALF], F16, name="gh16")
            gidx = sp.tile([2, 1], I32, name="gidx")
            parts = sp.tile([128, 2], F32, name="parts")
            pr = sp.tile([1, 2], F32, name="pr")
            gd = sp.tile([1, 48], F32, name="gd")
            sc = sp.tile([1, 16], F32, name="sc")
            ab = sp.tile([128, 2], F32, name="ab")         # [beta, alpha]
            ones_r = sp.tile([1, 128], F32, name="ones_r")
            ones_c = sp.tile([128, 1], F32, name="ones_c")

            XO = 0          # x at cols [0, W)
            RO = W          # r at cols [W, 2W)
            SO = 2 * W      # s~ at cols [2W, 3W)

            # ---------------- init ----------------
            nc.sync.dma_start(hb[:], b_in.ap())
            nc.sync.dma_start(gh16[:], gb_in.ap())
            nc.sync.dma_start(gidx[:], gidx_in.ap())
            nc.vector.memset(xrs[:], 0.0)
            nc.vector.memset(pq[:], 0.0)
            nc.vector.memset(s1[:], 0.0)
            nc.vector.memset(tt[:], 0.0)
            nc.vector.memset(up[:], 0.0)
            nc.vector.memset(dn[:], 0.0)
            nc.vector.memset(g[:], 0.0)
            nc.vector.memset(gs[:], 0.0)
            nc.vector.memset(sc[:], 0.0)
            nc.vector.memset(ones_r[:], 1.0)
            nc.vector.memset(ones_c[:], 1.0)
            # r0 = b (guards in b are zero); gr0 = b ghost rows
            nc.vector.tensor_copy(xrs[:, RO:RO + W], hb[:])
            nc.vector.tensor_copy(g[0:2, HALF:2 * HALF], gh16[:])

            for it in range(niter):
                ig_new = 2 + (it % 2)
                ig_old = 2 + ((it + 1) % 2)

                # ---- s~ = -A r = 4r - (rL + rR + rUp + rDn), Neumann ----
                nc.vector.tensor_tensor(
                    _int(s1, 0), _intsh(xrs, RO, -1), _intsh(xrs, RO, +1),
                    mybir.AluOpType.add)
                nc.sync.dma_start(up[1:128, :], xrs[0:127, RO:RO + W])
                nc.sync.dma_start(dn[0:127, :], xrs[1:128, RO:RO + W])
                nc.sync.dma_start(up[0:1, 2:2050], g[0:1, HALF:2 * HALF])
                nc.sync.dma_start(up[0:1, 2052:4100],
                                  xrs[127:128, RO + 2:RO + 2050])
                nc.sync.dma_start(dn[127:128, 2052:4100],
                                  g[1:2, HALF:2 * HALF])
                nc.sync.dma_start(dn[127:128, 2:2050],
                                  xrs[0:1, RO + 2052:RO + 4100])
                nc.vector.tensor_tensor(tt[:], s1[:], up[:],
                                        mybir.AluOpType.add)
                nc.vector.tensor_tensor(tt[:], tt[:], dn[:],
                                        mybir.AluOpType.add)
                nc.vector.scalar_tensor_tensor(
                    xrs[:, SO:SO + W], xrs[:, RO:RO + W], 4.0, tt[:],
                    mybir.AluOpType.mult, mybir.AluOpType.subtract)
                nc.vector.tensor_tensor(
                    _edge(xrs, SO), _edge(xrs, SO), _edge(xrs, RO),
                    mybir.AluOpType.subtract)

                # ---- dots: gam = r.r, dtn = r.s~ ----
                nc.vector.scalar_tensor_tensor(
                    up[:], xrs[:, RO:RO + W], 1.0, xrs[:, RO:RO + W],
                    mybir.AluOpType.mult, mybir.AluOpType.mult,
                    accum_out=parts[:, 0:1])
                nc.vector.scalar_tensor_tensor(
                    dn[:], xrs[:, RO:RO + W], 1.0, xrs[:, SO:SO + W],
                    mybir.AluOpType.mult, mybir.AluOpType.mult,
                    accum_out=parts[:, 1:2])
                red = ppb.tile([1, 2], F32, name="red", tag="red")
                nc.tensor.matmul(red[:], ones_c[:], parts[:],
                                 start=True, stop=True)
                nc.vector.tensor_copy(pr[0:1, 0:2], red[:])

                # ---- one AllGather: dots + boundary s~ rows ----
                nc.sync.dma_start(cc_in.ap()[0:1, 0:2], pr[0:1, 0:2])
                nc.sync.dma_start(cc_in.ap()[1:2, :],
                                  xrs[0:1, SO + 2:SO + 2050])
                nc.sync.dma_start(cc_in.ap()[2:3, :],
                                  xrs[127:128, SO + 2052:SO + 4100])
                nc.gpsimd.collective_compute(
                    "AllGather", mybir.AluOpType.bypass, replica_groups=rg,
                    ins=[cc_in.ap()], outs=[cc_out.ap()])
                nc.sync.dma_start(
                    gd[:].rearrange("a (k s) -> a k s", k=24),
                    cc_out.ap()[:, 0:2].rearrange("(o a) b -> o a b", o=1))
                nc.gpsimd.indirect_dma_start(
                    out=gs[:], out_offset=None, in_=cc_out.ap(),
                    in_offset=bass.IndirectOffsetOnAxis(ap=gidx[:, :1],
                                                        axis=0))

                # ---- reduce gathered dots; scalar recurrences ----
                gd3 = gd[:].rearrange("a (k s) -> a k s", k=8)
                nc.vector.tensor_reduce(
                    sc[:, 0:1], gd3[:, :, 0:1], axis=mybir.AxisListType.XY,
                    op=mybir.AluOpType.add)
                nc.vector.tensor_reduce(
                    sc[:, 1:2], gd3[:, :, 1:2], axis=mybir.AxisListType.XY,
                    op=mybir.AluOpType.add)
                # slots: 0 gam, 1 dtn, 2/3 invgam ping-pong, 4 beta, 5 alpha,
                #        6 u2, 7 v2, 8 w, 9 Nneg
                nc.vector.reciprocal(sc[:, ig_new:ig_new + 1], sc[:, 0:1])
                nc.vector.tensor_tensor(sc[:, 4:5], sc[:, 0:1],
                                        sc[:, ig_old:ig_old + 1],
                                        mybir.AluOpType.mult)
                nc.vector.tensor_tensor(sc[:, 6:7], sc[:, 1:2],
                                        sc[:, ig_new:ig_new + 1],
                                        mybir.AluOpType.mult)
                nc.vector.tensor_tensor(sc[:, 7:8], sc[:, 4:5], sc[:, 9:10],
                                        mybir.AluOpType.mult)
                nc.vector.tensor_tensor(sc[:, 8:9], sc[:, 7:8], sc[:, 6:7],
                                        mybir.AluOpType.subtract)
                nc.vector.reciprocal(sc[:, 5:6], sc[:, 8:9])
                nc.vector.tensor_tensor(sc[:, 9:10], sc[:, 6:7], sc[:, 7:8],
                                        mybir.AluOpType.subtract)
                bc = ppb.tile([128, 2], F32, name="bc", tag="bc")
                nc.tensor.matmul(bc[:], ones_r[:], sc[0:1, 4:6],
                                 start=True, stop=True)
                nc.vector.tensor_copy(ab[:], bc[:])

                # ---- updates: p,q~ then x,r; ghost rows likewise ----
                nc.vector.scalar_tensor_tensor(
                    pq[:], pq[:], ab[:, 0:1], xrs[:, RO:RO + 2 * W],
                    mybir.AluOpType.mult, mybir.AluOpType.add)
                nc.vector.scalar_tensor_tensor(
                    xrs[:, XO:XO + 2 * W], pq[:], ab[:, 1:2],
                    xrs[:, XO:XO + 2 * W],
                    mybir.AluOpType.mult, mybir.AluOpType.add)
                nc.vector.scalar_tensor_tensor(
                    g[0:2, 0:HALF], g[0:2, 0:HALF], ab[0:2, 0:1], gs[:],
                    mybir.AluOpType.mult, mybir.AluOpType.add)
                nc.vector.scalar_tensor_tensor(
                    g[0:2, HALF:2 * HALF], g[0:2, 0:HALF], ab[0:2, 1:2],
                    g[0:2, HALF:2 * HALF],
                    mybir.AluOpType.mult, mybir.AluOpType.add)

            # ---- output: strip guards, cast to f16 ----
            nc.vector.tensor_copy(
                of[:].rearrange("p (a w) -> p a w", a=2), _int(xrs, XO))
            nc.sync.dma_start(x_out.ap(), of[:])

    nc.compile()
    return nc


def _host_prep(conduit_size, discharge, geometric_gradient):
    f = np.float32
    cs = np.asarray(conduit_size, dtype=f).reshape(R, C)
    dc = np.asarray(discharge, dtype=f).reshape(R, C)
    gg = np.asarray(geometric_gradient, dtype=f).reshape(R, C)

    t = np.sqrt(cs)
    np.sqrt(t, out=t)
    t *= cs
    t *= dc
    t *= f(0.0405)
    np.square(t, out=t)          # gn

    gh = f(0.5) * (t[:, :-1] + t[:, 1:])
    gv = f(0.5) * (t[:-1, :] + t[1:, :])
    # boundary-link overrides (geometric gradient on links touching perimeter)
    gh[0, :] = f(0.5) * (gg[0, :-1] + gg[0, 1:])
    gh[-1, :] = f(0.5) * (gg[-1, :-1] + gg[-1, 1:])
    gh[:, 0] = f(0.5) * (gg[:, 0] + gg[:, 1])
    gh[:, -1] = f(0.5) * (gg[:, -2] + gg[:, -1])
    gv[0, :] = f(0.5) * (gg[0, :] + gg[1, :])
    gv[-1, :] = f(0.5) * (gg[-2, :] + gg[-1, :])
    gv[:, 0] = f(0.5) * (gg[:-1, 0] + gg[1:, 0])
    gv[:, -1] = f(0.5) * (gg[:-1, -1] + gg[1:, -1])

    b = np.zeros((R, C), dtype=f)
    b[:, :-1] += gh
    b[:, 1:] -= gh
    b[:-1, :] += gv
    b[1:, :] -= gv
    return b, gg


def kernel(conduit_size, discharge, geometric_gradient, nrows, ncols):
    global _compiled
    b, gg = _host_prep(conduit_size, discharge, geometric_gradient)

    if _compiled is None:
        _compiled = _build_program(NITER)
    nc = _compiled

    in_maps = []
    for i in range(NCORES):
        bb = np.zeros((128, W), dtype=np.float16)
        bb[:, 2:2050] = b[i * BR:i * BR + 128, :]
        bb[:, 2052:4100] = b[i * BR + 128:(i + 1) * BR, :]
        gb = np.zeros((2, HALF), dtype=np.float16)
        gb[0, :] = b[i * BR - 1, :] if i > 0 else b[0, :]
        gb[1, :] = b[(i + 1) * BR, :] if i < NCORES - 1 else b[R - 1, :]
        lo = 3 * (i - 1) + 2 if i > 0 else 1
        hi = 3 * (i + 1) + 1 if i < NCORES - 1 else 3 * (NCORES - 1) + 2
        gidx = np.array([[lo], [hi]], dtype=np.int32)
        in_maps.append({"bblk": bb, "gb": gb, "gidx": gidx})

    res = run_bass_kernel_spmd(nc, in_maps, core_ids=list(range(NCORES)))

    out = gg  # gg is already a private f32 copy from _host_prep
    for i in range(NCORES):
        xf = res.results[i]["xout"].astype(np.float32)
        out[i * BR:i * BR + 128, :] -= np.float32(DX) * xf[:, 0:HALF]
        out[i * BR + 128:(i + 1) * BR, :] -= np.float32(DX) * xf[:, HALF:]
    return out.reshape(-1)


# revision 7
# speedup vs baseline: 1.6867x; 1.0371x over previous
"""Trainium2 Bass kernel for nn_ConduitHydrology: 100 iterations of
single-reduction (Chronopoulos-Gear) CG on the 5-point Neumann Laplacian,
2048x2048 raster, row-blocks over 8 cores, f16 I/O.

kernel(**inputs) takes FULL inputs and returns the FULL output.
"""
import numpy as np

import concourse.bass as bass
import concourse.bacc as bacc
import concourse.mybir as mybir
import concourse.tile as tile
from concourse.bass_utils import run_bass_kernel_spmd
from concourse import bass_isa

F32 = mybir.dt.float32
F16 = mybir.dt.float16
I32 = mybir.dt.int32
NCORES = 8
R, C = 2048, 2048
BR = R // NCORES          # 256 grid rows per core, packed as 2 slabs of 128
HALF = 2048
W = 4104                  # [gg | s0(2048) | gg | s1(2048) | gggg]
NITER = 100
DX = 100.0

_compiled = None


def _int(t, off):
    """Nested AP covering both slab interiors of a width-W region at column
    offset `off` within the tile: cols {off+2..off+2049} u {off+2052..off+4099}."""
    return t[:, off + 2:off + 4102].rearrange("p (a w) -> p a w", a=2)[:, :, 0:2048]


def _intsh(t, off, sh):
    """Interior nested AP shifted by sh (-1 left neighbors, +1 right)."""
    base = off + 2 + sh
    return t[:, base:base + 4100].rearrange("p (a w) -> p a w", a=2)[:, :, 0:2048]


def _edge(t, off):
    """The 4 slab-edge columns {off+2, off+2049, off+2052, off+4099}."""
    return t[:, off + 2:off + 4102].rearrange(
        "p (a w) -> p a w", a=2)[:, :, 0:2048:2047]


def _build_program(niter):
    nc = bacc.Bacc("TRN2", target_bir_lowering=False, debug=False,
                   num_devices=NCORES)

    b_in = nc.dram_tensor("bblk", [128, W], F16, kind="ExternalInput")
    gb_in = nc.dram_tensor("gb", [2, HALF], F16, kind="ExternalInput")
    gidx_in = nc.dram_tensor("gidx", [2, 1], I32, kind="ExternalInput")
    x_out = nc.dram_tensor("xout", [128, 2 * HALF], F16, kind="ExternalOutput")

    cc_in = nc.dram_tensor("cc_in", [3, HALF], F32, kind="Internal")
    cc_out = nc.dram_tensor("cc_out", [3 * NCORES, HALF], F32,
                            kind="Internal", addr_space="Shared")
    rg = [list(range(NCORES))]

    with tile.TileContext(nc) as tc:
        with tc.tile_pool(name="state", bufs=1) as sp, \
             tc.tile_pool(name="psumB", bufs=1, space="PSUM") as ppb:
            # xrs = [x | r | s~] with s~ = -A r; pq = [p | q~], q~ = -A p
            xrs = sp.tile([128, 3 * W], F32, name="xrs")
            pq = sp.tile([128, 2 * W], F32, name="pq")
            s1 = sp.tile([128, W], F32, name="s1")
            tt = sp.tile([128, W], F32, name="tt")
            up = sp.tile([128, W], F32, name="up")
            dn = sp.tile([128, W], F32, name="dn")
            hb = sp.tile([128, W], F16, name="hb")
            of = sp.tile([128, 2 * HALF], F16, name="of")
            g = sp.tile([2, 2 * HALF], F32, name="g")      # [gq~ | gr]
            gs = sp.tile([2, HALF], F32, name="gs")        # ghost s~ rows
            gh16 = sp.tile([2, HALF], F16, name="gh16")
            gidx = sp.tile([2, 1], I32, name="gidx")
            parts = sp.tile([128, 2], F32, name="parts")
            pr = sp.tile([1, 2], F32, name="pr")
            gd = sp.tile([1, 48], F32, name="gd")
            sc = sp.tile([1, 16], F32, name="sc")
            ab = sp.tile([128, 2], F32, name="ab")         # [beta, alpha]
            ones_r = sp.tile([1, 128], F32, name="ones_r")
            ones_c = sp.tile([128, 1], F32, name="ones_c")

            XO = 0          # x at cols [0, W)
            RO = W          # r at cols [W, 2W)
            SO = 2 * W      # s~ at cols [2W, 3W)

            # ---------------- init ----------------
            nc.sync.dma_start(hb[:], b_in.ap())
            nc.sync.dma_start(gh16[:], gb_in.ap())
            nc.sync.dma_start(gidx[:], gidx_in.ap())
            nc.vector.memset(xrs[:], 0.0)
            nc.vector.memset(pq[:], 0.0)
            nc.vector.memset(s1[:], 0.0)
            nc.vector.memset(tt[:], 0.0)
            nc.vector.memset(up[:], 0.0)
            nc.vector.memset(dn[:], 0.0)
            nc.vector.memset(g[:], 0.0)
            nc.vector.memset(gs[:], 0.0)
            nc.vector.memset(sc[:], 0.0)
            nc.vector.memset(ones_r[:], 1.0)
            nc.vector.memset(ones_c[:], 1.0)
            # r0 = b (guards in b are zero); gr0 = b ghost rows
            nc.vector.tensor_copy(xrs[:, RO:RO + W], hb[:])
            nc.vector.tensor_copy(g[0:2, HALF:2 * HALF], gh16[:])

            for it in range(niter):
                ig_new = 2 + (it % 2)
                ig_old = 2 + ((it + 1) % 2)

                # ---- s~ = -A r = 4r - (rL + rR + rUp + rDn), Neumann ----
                nc.vector.tensor_tensor(
                    _int(s1, 0), _intsh(xrs, RO, -1), _intsh(xrs, RO, +1),
                    mybir.AluOpType.add)
                nc.sync.dma_start(up[1:128, :], xrs[0:127, RO:RO + W])
                nc.sync.dma_start(dn[0:127, :], xrs[1:128, RO:RO + W])
                nc.sync.dma_start(up[0:1, 2:2050], g[0:1, HALF:2 * HALF])
                nc.sync.dma_start(up[0:1, 2052:4100],
                                  xrs[127:128, RO + 2:RO + 2050])
                nc.sync.dma_start(dn[127:128, 2052:4100],
                                  g[1:2, HALF:2 * HALF])
                nc.sync.dma_start(dn[127:128, 2:2050],
                                  xrs[0:1, RO + 2052:RO + 4100])
                nc.vector.tensor_tensor(tt[:], s1[:], up[:],
                                        mybir.AluOpType.add)
                nc.vector.tensor_tensor(tt[:], tt[:], dn[:],
                                        mybir.AluOpType.add)
                nc.vector.scalar_tensor_tensor(
                    xrs[:, SO:SO + W], xrs[:, RO:RO + W], 4.0, tt[:],
                    mybir.AluOpType.mult, mybir.AluOpType.subtract)
                nc.vector.tensor_tensor(
                    _edge(xrs, SO), _edge(xrs, SO), _edge(xrs, RO),
                    mybir.AluOpType.subtract)

                # ---- dots: gam = r.r, dtn = r.s~ ----
                nc.vector.scalar_tensor_tensor(
                    up[:], xrs[:, RO:RO + W], 1.0, xrs[:, RO:RO + W],
                    mybir.AluOpType.mult, mybir.AluOpType.mult,
                    accum_out=parts[:, 0:1])
                nc.vector.scalar_tensor_tensor(
                    dn[:], xrs[:, RO:RO + W], 1.0, xrs[:, SO:SO + W],
                    mybir.AluOpType.mult, mybir.AluOpType.mult,
                    accum_out=parts[:, 1:2])
                red = ppb.tile([1, 2], F32, name="red", tag="red")
                nc.tensor.matmul(red[:], ones_c[:], parts[:],
                                 start=True, stop=True)
                nc.vector.tensor_copy(pr[0:1, 0:2], red[:])

                # ---- one AllGather: dots + boundary s~ rows ----
                nc.sync.dma_start(cc_in.ap()[0:1, 0:2], pr[0:1, 0:2])
                nc.sync.dma_start(cc_in.ap()[1:2, :],
                                  xrs[0:1, SO + 2:SO + 2050])
                nc.sync.dma_start(cc_in.ap()[2:3, :],
                                  xrs[127:128, SO + 2052:SO + 4100])
                nc.gpsimd.collective_compute(
                    "AllGather", mybir.AluOpType.bypass, replica_groups=rg,
                    ins=[cc_in.ap()], outs=[cc_out.ap()])
                nc.sync.dma_start(
                    gd[:].rearrange("a (k s) -> a k s", k=24),
                    cc_out.ap()[:, 0:2].rearrange("(o a) b -> o a b", o=1))
                nc.gpsimd.indirect_dma_start(
                    out=gs[:], out_offset=None, in_=cc_out.ap(),
                    in_offset=bass.IndirectOffsetOnAxis(ap=gidx[:, :1],
                                                        axis=0))

                # ---- reduce gathered dots; scalar recurrences ----
                gd3 = gd[:].rearrange("a (k s) -> a k s", k=8)
                nc.vector.tensor_reduce(
                    sc[:, 0:1], gd3[:, :, 0:1], axis=mybir.AxisListType.XY,
                    op=mybir.AluOpType.add)
                nc.vector.tensor_reduce(
                    sc[:, 1:2], gd3[:, :, 1:2], axis=mybir.AxisListType.XY,
                    op=mybir.AluOpType.add)
                # slots: 0 gam, 1 dtn, 2/3 invgam ping-pong, 4 beta, 5 alpha,
                #        6 u2, 7 v2, 8 w, 9 Nneg
                nc.vector.reciprocal(sc[:, ig_new:ig_new + 1], sc[:, 0:1])
                nc.vector.tensor_tensor(sc[:, 4:5], sc[:, 0:1],
                                        sc[:, ig_old:ig_old + 1],
                                        mybir.AluOpType.mult)
                nc.vector.tensor_tensor(sc[:, 6:7], sc[:, 1:2],
                                        sc[:, ig_new:ig_new + 1],
                                        mybir.AluOpType.mult)
                nc.vector.tensor_tensor(sc[:, 7:8], sc[:, 4:5], sc[:, 9:10],
                                        mybir.AluOpType.mult)
                nc.vector.tensor_tensor(sc[:, 8:9], sc[:, 7:8], sc[:, 6:7],
                                        mybir.AluOpType.subtract)
                nc.vector.reciprocal(sc[:, 5:6], sc[:, 8:9])
                nc.vector.tensor_tensor(sc[:, 9:10], sc[:, 6:7], sc[:, 7:8],
                                        mybir.AluOpType.subtract)
                bc = ppb.tile([128, 2], F32, name="bc", tag="bc")
                nc.tensor.matmul(bc[:], ones_r[:], sc[0:1, 4:6],
                                 start=True, stop=True)
                nc.vector.tensor_copy(ab[:], bc[:])

                # ---- updates: p,q~ then x,r; ghost rows likewise ----
                nc.vector.scalar_tensor_tensor(
                    pq[:], pq[:], ab[:, 0:1], xrs[:, RO:RO + 2 * W],
                    mybir.AluOpType.mult, mybir.AluOpType.add)
                nc.vector.scalar_tensor_tensor(
                    xrs[:, XO:XO + 2 * W], pq[:], ab[:, 1:2],
                    xrs[:, XO:XO + 2 * W],
                    mybir.AluOpType.mult, mybir.AluOpType.add)
                nc.vector.scalar_tensor_tensor(
                    g[0:2, 0:HALF], g[0:2, 0:HALF], ab[0:2, 0:1], gs[:],
                    mybir.AluOpType.mult, mybir.AluOpType.add)
                nc.vector.scalar_tensor_tensor(
                    g[0:2, HALF:2 * HALF], g[0:2, 0:HALF], ab[0:2, 1:2],
                    g[0:2, HALF:2 * HALF],
                    mybir.AluOpType.mult, mybir.AluOpType.add)

            # ---- output: strip guards, cast to f16 ----
            nc.vector.tensor_copy(
                of[:].rearrange("p (a w) -> p a w", a=2), _int(xrs, XO))
            nc.sync.dma_start(x_out.ap(), of[:])

    nc.compile()
    return nc


def _host_prep(conduit_size, discharge, geometric_gradient):
    f = np.float32
    cs = np.asarray(conduit_size, dtype=f).reshape(R, C)
    dc = np.asarray(discharge, dtype=f).reshape(R, C)
    gg = np.asarray(geometric_gradient, dtype=f).reshape(R, C)

    t = np.sqrt(cs)
    np.sqrt(t, out=t)
    t *= cs
    t *= dc
    t *= f(0.0405)
    np.square(t, out=t)          # gn

    gh = f(0.5) * (t[:, :-1] + t[:, 1:])
    gv = f(0.5) * (t[:-1, :] + t[1:, :])
    # boundary-link overrides (geometric gradient on links touching perimeter)
    gh[0, :] = f(0.5) * (gg[0, :-1] + gg[0, 1:])
    gh[-1, :] = f(0.5) * (gg[-1, :-1] + gg[-1, 1:])
    gh[:, 0] = f(0.5) * (gg[:, 0] + gg[:, 1])
    gh[:, -1] = f(0.5) * (gg[:, -2] + gg[:, -1])
    gv[0, :] = f(0.5) * (gg[0, :] + gg[1, :])
    gv[-1, :] = f(0.5) * (gg[-2, :] + gg[-1, :])
    gv[:, 0] = f(0.5) * (gg[:-1, 0] + gg[1:, 0])
    gv[:, -1] = f(0.5) * (gg[:-1, -1] + gg[1:, -1])

    b = np.zeros((R, C), dtype=f)
    b[:, :-1] += gh
    b[:, 1:] -= gh
    b[:-1, :] += gv
    b[1:, :] -= gv
    return b, gg


def kernel(conduit_size, discharge, geometric_gradient, nrows, ncols):
    global _compiled
    b, gg = _host_prep(conduit_size, discharge, geometric_gradient)

    if _compiled is None:
        _compiled = _build_program(NITER)
    nc = _compiled

    bb_all = np.zeros((NCORES, 128, W), dtype=np.float16)
    b4 = b.reshape(NCORES, 2, 128, C)
    bb_all[:, :, 2:2050] = b4[:, 0]
    bb_all[:, :, 2052:4100] = b4[:, 1]
    gb_all = np.empty((NCORES, 2, HALF), dtype=np.float16)
    gb_all[0, 0] = b[0]
    gb_all[1:, 0] = b[BR - 1::BR][:NCORES - 1]
    gb_all[:-1, 1] = b[BR::BR][:NCORES - 1]
    gb_all[-1, 1] = b[R - 1]
    in_maps = []
    for i in range(NCORES):
        lo = 3 * (i - 1) + 2 if i > 0 else 1
        hi = 3 * (i + 1) + 1 if i < NCORES - 1 else 3 * (NCORES - 1) + 2
        gidx = np.array([[lo], [hi]], dtype=np.int32)
        in_maps.append({"bblk": bb_all[i], "gb": gb_all[i], "gidx": gidx})

    res = run_bass_kernel_spmd(nc, in_maps, core_ids=list(range(NCORES)))

    out = gg  # gg is already a private f32 copy from _host_prep
    for i in range(NCORES):
        xf = res.results[i]["xout"].astype(np.float32)
        out[i * BR:i * BR + 128, :] -= np.float32(DX) * xf[:, 0:HALF]
        out[i * BR + 128:(i + 1) * BR, :] -= np.float32(DX) * xf[:, HALF:]
    return out.reshape(-1)
